# revision 13
# baseline (speedup 1.0000x reference)
"""Trainium2 Bass kernel for nn_Attention_1013612281902.

Reference computation (per batch b, head h):
    Q = emb @ Wq[h].T            [S,C]
    K = emb_all @ Wk[h].T        [S,KV]
    V = emb_all @ Wv[h].T        [S,KV]
    scores = Q.T @ K / sqrt(KV)  [C,KV]
    normed = instance_norm(scores)       (mean/var over the whole [C,KV] plane)
    probs  = softmax(normed, axis=KV)
    context = probs @ V.T        [C,S]
    out = mean_h(context).T @ Wo.T       [S,C]

Algebraic restructuring (S=4096 >> C=512, KV=960):
    G = emb.T @ emb_all                      [C,KV]   (shared across heads)
    scores = (Wq[h] @ G @ Wk[h].T)/sqrt(KV)
    Pv[h]  = probs[h] @ Wv[h]                [C,KV]
    out    = emb_all @ (mean_h Pv[h]).T @ (Wo.T/4)
This avoids materializing Q/K/V entirely and cuts FLOPs ~4x.

All matmul operands are bfloat16 (fp32 PSUM): halves HBM traffic, enables
fast-weight-load so LDWEIGHTS hides in the PE reorder window, and runs PE
transposes at 1 cycle/row. Plane stats are computed from f32 PSUM scores;
the tiny [128,16] cross-partition stats matmul stays float32r. The Pv
matmuls put probs chunks stationary so Pv lands directly as [c,kv]
(= Pbar layout for the output projection -- no Pbar transposes), and Wv
carries an appended ones-column so each softmax denominator d[c] falls
out of the same accumulation for free. Emission order hides each head's
serial stats/exp chain under the other head's matmuls. End-to-end rel
err ~5e-3 vs the 2e-2 budget.

Sharding: 8 cores = (4 batches) x (2 head-pairs). Core 2b+g computes the
partial output for batch b over heads {2g, 2g+1}; the host adds the two
partials per batch (the head-mean and output projection are linear).
"""

import sys

if "/opt/trn_rl_repo" not in sys.path:
    sys.path.insert(0, "/opt/trn_rl_repo")

from contextlib import ExitStack

import numpy as np
import ml_dtypes

import concourse.bacc as bacc
import concourse.mybir as mybir
import concourse.tile as tile
from concourse.bass_utils import run_bass_kernel_spmd
from concourse.masks import make_identity
from concourse.tile_rust import add_dep_helper

B, S, C, KV, H = 4, 4096, 512, 960, 4
KVA = 968               # Wv free width: col 960 = ones (denominator), 961+ pad
EPS = 1e-5
F32 = mybir.dt.float32
F32R = mybir.dt.float32r
BF16 = mybir.dt.bfloat16

ST = S // 128           # 32 s-tiles
CT = C // 128           # 4 c-tiles
KT = (KV + 127) // 128  # 8 k-tiles (last one has 64 partitions)


def _kp(t):
    return min(128, KV - t * 128)


def _build_program():
    nc = bacc.Bacc("TRN2", target_bir_lowering=False, debug=False, num_devices=8)

    emb_d = nc.dram_tensor("emb", [S, C], BF16, kind="ExternalInput")
    ea_d = nc.dram_tensor("ea", [S, KV], BF16, kind="ExternalInput")
    wqt_d = nc.dram_tensor("wqt", [2, C, C], BF16, kind="ExternalInput")
    wkt_d = nc.dram_tensor("wkt", [2, KV, KV], BF16, kind="ExternalInput")
    wv_d = nc.dram_tensor("wv", [2, KV, KVA], BF16, kind="ExternalInput")
    wot_d = nc.dram_tensor("wot", [C, C], BF16, kind="ExternalInput")
    y_d = nc.dram_tensor("y", [S, C], F32, kind="ExternalOutput")

    with tile.TileContext(nc) as tc, ExitStack() as ectx:
        ec = ectx.enter_context
        const = ec(tc.tile_pool(name="const", bufs=1))
        gp = ec(tc.tile_pool(name="gp", bufs=1))
        # bufs=2 so BOTH heads' weights stream in as soon as the Sync queue
        # reaches them; with bufs=1 head-1's loads stall until head-0's
        # weights are consumed (~150us in), starving S1/W1.
        wqp = ec(tc.tile_pool(name="wqp", bufs=2))
        wkp = ec(tc.tile_pool(name="wkp", bufs=2))
        wvp = ec(tc.tile_pool(name="wvp", bufs=2))
        wop = ec(tc.tile_pool(name="wop", bufs=1))
        embp = ec(tc.tile_pool(name="embp", bufs=8))
        eap = ec(tc.tile_pool(name="eap", bufs=5))
        bigp = ec(tc.tile_pool(name="bigp", bufs=1))   # a_sb0/a_sb1/z_sb in turn
        scp = ec(tc.tile_pool(name="scp", bufs=2))     # scoresT -> probsT per head
        pbp = ec(tc.tile_pool(name="pbp", bufs=1))     # Pbar accumulator
        trp = ec(tc.tile_pool(name="trp", bufs=24))
        outp = ec(tc.tile_pool(name="outp", bufs=4))
        stp = ec(tc.tile_pool(name="stp", bufs=4))     # small stats tiles

        ident = const.tile([128, 128], BF16)
        make_identity(nc, ident[:])
        # f32r stats operand: the [128,16] cross-partition stats matmul needs
        # full fp32 precision (bf16 sums would feed var with ~0.4% error
        # straight into the softmax argument).
        onesf = const.tile([128, 128], F32)
        nc.vector.memset(onesf[:], 1.0)
        ones_r = const.tile([128, 128], F32R)
        nc.vector.tensor_copy(out=ones_r[:], in_=onesf[:])
        # scores are left unscaled (instance-norm is scale-invariant), so the
        # reference's eps applies to var/KV: use KV*eps against unscaled var.
        eps_t = const.tile([128, 1], F32)
        nc.vector.memset(eps_t[:], EPS * KV)
        # ACT-table prewarm scratch (Sqrt/Exp table loads are ~1.3us; a dummy
        # op issued early moves the load off the critical chain).
        warm = const.tile([128, 1], F32)
        nc.vector.memset(warm[:], 1.0)

        def prewarm(func, nm):
            wsink = stp.tile([128, 1], F32, tag="wsink", name=nm)
            nc.scalar.activation(out=wsink[:], in_=warm[:], func=func)

        # ---- phase 1: G = emb.T @ emb_all  [C, KV] --------------------------
        g_sb = gp.tile([128, CT, KV], BF16)
        gps_pool = tc.tile_pool(name="gps", bufs=8, space="PSUM")
        ps = gps_pool.__enter__()
        g_ps = [ps.tile([128, 480], F32, tag="ps", name=f"g_ps{i}") for i in range(8)]
        for st in range(ST):
            et = embp.tile([128, C], BF16, tag="emb", name=f"et{st}")
            at = eap.tile([128, KV], BF16, tag="ea", name=f"at{st}")
            rs = slice(st * 128, (st + 1) * 128)
            if st == 0:
                # split the first tile's loads so the first matmul (which
                # needs only et[:,0:128] + at[:,0:480]) starts ASAP.
                nc.sync.dma_start(out=at[:, 0:480], in_=ea_d.ap()[rs, 0:480])
                nc.sync.dma_start(out=et[:, 0:128], in_=emb_d.ap()[rs, 0:128])
                nc.sync.dma_start(out=et[:, 128:C], in_=emb_d.ap()[rs, 128:C])
                nc.sync.dma_start(out=at[:, 480:KV], in_=ea_d.ap()[rs, 480:KV])
            else:
                nc.sync.dma_start(out=et[:], in_=emb_d.ap()[rs, :])
                nc.sync.dma_start(out=at[:], in_=ea_d.ap()[rs, :])
            for ct in range(CT):
                for kc in range(2):
                    nc.tensor.matmul(
                        g_ps[ct * 2 + kc][:],
                        et[:, ct * 128 : (ct + 1) * 128],
                        at[:, kc * 480 : (kc + 1) * 480],
                        start=(st == 0),
                        stop=(st == ST - 1),
                    )
        for ct in range(CT):
            for kc in range(2):
                # Alternate ACT/DVE so the copy-out tail after the last G
                # matmul drains in half the time.
                dst = g_sb[:, ct, kc * 480 : (kc + 1) * 480]
                if (ct * 2 + kc) % 2 == 0:
                    nc.vector.tensor_copy(out=dst, in_=g_ps[ct * 2 + kc][:])
                else:
                    nc.scalar.copy(out=dst, in_=g_ps[ct * 2 + kc][:])
        gps_pool.__exit__(None, None, None)

        # ---- weights (host provides pre-transposed Wq.T / Wk.T / Wo.T/4) ----
        # Issued after the G-phase streams so the emb/emb_all DMAs (which
        # gate the first matmuls) get the HBM bandwidth first; within the
        # weights, in consumption order (wqt0 gates phase 2a).
        wqt_sb = []
        wkt_sb = []
        wv_sb = []
        for h in range(2):
            wq_t = wqp.tile([128, CT, C], BF16, tag="wq", name=f"wq{h}")
            nc.sync.dma_start(
                out=wq_t[:],
                in_=wqt_d.ap()[h].rearrange("(t p) d -> p t d", p=128),
            )
            wqt_sb.append(wq_t)
            # 960 rows = 7x128 + 64: two DMAs per tensor (fewer dma_starts --
            # each costs ~700ns of serial Sync-engine issue time).
            wk_t = wkp.tile([128, KT, KV], BF16, tag="wk", name=f"wk{h}")
            wv_t = wvp.tile([128, KT, KVA], BF16, tag="wv", name=f"wv{h}")
            nc.sync.dma_start(
                out=wk_t[:, 0:7, :],
                in_=wkt_d.ap()[h, 0:896, :].rearrange("(t p) d -> p t d", p=128),
            )
            nc.sync.dma_start(
                out=wk_t[:64, 7, :], in_=wkt_d.ap()[h, 896:KV, :]
            )
            nc.sync.dma_start(
                out=wv_t[:, 0:7, :],
                in_=wv_d.ap()[h, 0:896, :].rearrange("(t p) d -> p t d", p=128),
            )
            nc.sync.dma_start(
                out=wv_t[:64, 7, :], in_=wv_d.ap()[h, 896:KV, :]
            )
            wkt_sb.append(wk_t)
            wv_sb.append(wv_t)
        wot_sb = wop.tile([128, CT, C], BF16)
        nc.sync.dma_start(
            out=wot_sb[:], in_=wot_d.ap().rearrange("(t p) d -> p t d", p=128)
        )

        # ---- phase 2: per-head scores -> instancenorm -> softmax -> Pv ------
        # Emission order: A0 S0 stats0 A1 S1(+exp0 injected) waves0
        # stats1+exps1 waves1. Each head's serial stats/exp chain runs on
        # DVE/ACT under the other head's (or its own waves') PE matmuls, so
        # the PE stream A0 S0 A1 S1 W0 W1 never waits on it. One PSUM pool,
        # 8 banks: psa(2, A groups + wave_B ct0/1) + pw(4, scoresT groups +
        # wave_A + wave_B ct2/3) + one(2, stats).
        pbar_sb = pbp.tile([128, CT, KV], BF16)
        ph2_pool = tc.tile_pool(name="ph2ps", bufs=1, space="PSUM")
        ps = ph2_pool.__enter__()
        hs = [{}, {}]

        def emit_A(h):
            d = hs[h]
            d["a_sb"] = a_sb = bigp.tile(
                [128, KT, C], BF16, tag="big", name=f"a_sb{h}"
            )
            for kt in range(KT):
                kp = _kp(kt)
                pa = ps.tile([128, C], F32, tag="psa", bufs=2, name=f"pa{h}{kt}")
                for ct in range(CT):
                    nc.tensor.matmul(
                        pa[:kp, :],
                        g_sb[:, ct, kt * 128 : kt * 128 + kp],
                        wqt_sb[h][:, ct, :],
                        start=(ct == 0),
                        stop=(ct == CT - 1),
                    )
                nc.vector.tensor_copy(out=a_sb[:kp, kt, :], in_=pa[:kp, :])

        def emit_scoresT(h, inject=None):
            # scoresT[j, d] = sum_k WkT[k,j] A.T[k,d]; the reference's
            # 1/sqrt(KV) scale cancels through instance-norm (eps adjusted).
            # Per-jt stats partials (row-sum on DVE, square-sum via an
            # in-place DVE multiply-reduce -- no ACT Square table) run right
            # behind each group; `inject` emits the other head's exp ops
            # into the ACT stream so they hide under this head's matmuls.
            d = hs[h]
            a_sb = d["a_sb"]
            d["sc_sb"] = sc_sb = scp.tile(
                [128, KT, C], BF16, tag="sc", name=f"sc_sb{h}"
            )
            d["p_sb"] = p_sb = stp.tile([128, 16], F32, tag="p16", name=f"p_sb{h}")
            nc.vector.memset(p_sb[:], 0.0)
            prev_stop = None
            for jt in range(KT):
                jp = _kp(jt)
                pss = ps.tile([128, C], F32, tag="pw", bufs=4, name=f"pss{h}{jt}")
                for kt in range(KT):
                    kp = _kp(kt)
                    mm = nc.tensor.matmul(
                        pss[:jp, :],
                        wkt_sb[h][:kp, kt, jt * 128 : jt * 128 + jp],
                        a_sb[:kp, kt, :],
                        start=(kt == 0),
                        stop=(kt == KT - 1),
                    )
                    # Keep the PE stream jt-group-major: otherwise the
                    # scheduler interleaves the groups and every stop lands
                    # at the tail, stalling the stats.
                    if kt == 0 and prev_stop is not None:
                        add_dep_helper(
                            mm.ins, prev_stop.ins, sync=False, reason="jt order"
                        )
                    if kt == KT - 1:
                        prev_stop = mm
                # copy-out fused with the row-sum on ACT (Copy needs no table,
                # so the other head's exps interleave freely); square-sum as
                # pss * sc_sb (f32 PSUM x its bf16 copy) on the DVE.
                nc.scalar.activation(
                    out=sc_sb[:jp, jt, :],
                    in_=pss[:jp, :],
                    func=mybir.ActivationFunctionType.Copy,
                    accum_out=p_sb[:jp, jt : jt + 1],
                )
                sq_sink = stp.tile(
                    [128, C], BF16, tag="sqsink", name=f"sqs{h}{jt}"
                )
                nc.vector.tensor_mul(
                    out=sq_sink[:jp, :],
                    in0=pss[:jp, :],
                    in1=sc_sb[:jp, jt, :],
                )
                nc.vector.reduce_sum(
                    out=p_sb[:jp, 8 + jt : 9 + jt],
                    in_=sq_sink[:jp, :],
                    axis=mybir.AxisListType.X,
                )
                if inject is not None:
                    inject(jt)

        def emit_stats(h):
            # cross-partition reduce + broadcast of the plane stats, ending
            # in rstd / -mean*rstd for the fused exp. Runs under the next
            # phase's matmuls; prewarm(Exp) drags the table load off-chain.
            d = hs[h]
            p_sb = d["p_sb"]
            p_r = stp.tile([128, 16], F32R, tag="p16r", name=f"p_r{h}")
            nc.vector.tensor_copy(out=p_r[:], in_=p_sb[:])
            pst = ps.tile([128, 16], F32, tag="one", bufs=2, name=f"pst{h}")
            nc.tensor.matmul(pst[:], ones_r[:], p_r[:], start=True, stop=True)
            n_inv = 1.0 / float(C * KV)
            sq2 = stp.tile([128, 2], F32, tag="sq2", name=f"sq2{h}")
            nc.vector.reduce_sum(
                out=sq2[:],
                in_=pst[:].rearrange("p (a b) -> p a b", a=2),
                axis=mybir.AxisListType.X,
            )
            mean_neg = stp.tile([128, 1], F32, tag="mean", name=f"mean{h}")
            nc.vector.tensor_scalar(
                out=mean_neg[:], in0=sq2[:, 0:1], scalar1=-n_inv, scalar2=None,
                op0=mybir.AluOpType.mult,
            )
            em2 = stp.tile([128, 1], F32, tag="em2", name=f"em2{h}")
            nc.vector.tensor_scalar(
                out=em2[:], in0=sq2[:, 1:2], scalar1=n_inv, scalar2=None,
                op0=mybir.AluOpType.mult,
            )
            m2 = stp.tile([128, 1], F32, tag="m2", name=f"m2{h}")
            nc.vector.tensor_mul(out=m2[:], in0=mean_neg[:], in1=mean_neg[:])
            var_t = stp.tile([128, 1], F32, tag="var", name=f"var{h}")
            nc.vector.tensor_sub(out=var_t[:], in0=em2[:], in1=m2[:])
            std_t = stp.tile([128, 1], F32, tag="std", name=f"std{h}")
            nc.scalar.activation(
                out=std_t[:],
                in_=var_t[:],
                func=mybir.ActivationFunctionType.Sqrt,
                bias=eps_t[:],
            )
            prewarm(mybir.ActivationFunctionType.Exp, f"wex{h}")
            rstd_t = stp.tile([128, 1], F32, tag="rstd", name=f"rstd{h}")
            nc.vector.reciprocal(out=rstd_t[:], in_=std_t[:])
            negmr = stp.tile([128, 1], F32, tag="negmr", name=f"negmr{h}")
            nc.vector.tensor_mul(out=negmr[:], in0=mean_neg[:], in1=rstd_t[:])
            d["rstd"] = rstd_t
            d["negmr"] = negmr

        def emit_exp(h, jt):
            d = hs[h]
            jp = _kp(jt)
            nc.scalar.activation(
                out=d["sc_sb"][:jp, jt, :],
                in_=d["sc_sb"][:jp, jt, :],
                func=mybir.ActivationFunctionType.Exp,
                bias=d["negmr"][:jp],
                scale=d["rstd"][:jp],
            )

        def emit_waves(h):
            # Pv with probs chunks stationary: pp[ct] = sum_jt
            # probsT[jt,ct-chunk].T @ Wv-rows[jt, slice]  ->  Pv[c, kv].
            # wave_A covers kv 480:960 plus the ones column, so pp_A[:,480]
            # is the softmax denominator d[c]; its reciprocal scales every
            # copy-out. ct-major groups: d[ct] is ready as soon as group ct
            # stops, so copy-outs overlap the remaining groups.
            d = hs[h]
            sc_sb = d["sc_sb"]
            r4c = stp.tile([128, 4], F32, tag="r4c", name=f"r4c{h}")
            pp_A = []
            for ct in range(CT):
                pp = ps.tile([128, 488], F32, tag="pw", bufs=4, name=f"pA{h}{ct}")
                for jt in range(KT):
                    jp = _kp(jt)
                    nc.tensor.matmul(
                        pp[:, :],
                        sc_sb[:jp, jt, ct * 128 : (ct + 1) * 128],
                        wv_sb[h][:jp, jt, 480:KVA],
                        start=(jt == 0),
                        stop=(jt == KT - 1),
                    )
                pp_A.append(pp)
                nc.vector.reciprocal(out=r4c[:, ct : ct + 1], in_=pp[:, 480:481])
                if ct >= 2:
                    pv_out(h, ct - 2, pp_A[ct - 2], r4c, 1)
            pv_out(h, 2, pp_A[2], r4c, 1)
            pv_out(h, 3, pp_A[3], r4c, 1)
            pp_B = []
            for ct in range(CT):
                tag = "psa" if ct < 2 else "pw"
                bufs = 2 if ct < 2 else 4
                pp = ps.tile([128, 480], F32, tag=tag, bufs=bufs, name=f"pB{h}{ct}")
                for jt in range(KT):
                    jp = _kp(jt)
                    nc.tensor.matmul(
                        pp[:, :],
                        sc_sb[:jp, jt, ct * 128 : (ct + 1) * 128],
                        wv_sb[h][:jp, jt, 0:480],
                        start=(jt == 0),
                        stop=(jt == KT - 1),
                    )
                pp_B.append(pp)
                if ct >= 1:
                    pv_out(h, ct - 1, pp_B[ct - 1], r4c, 0)
            pv_out(h, 3, pp_B[3], r4c, 0)

        def pv_out(h, ct, pp, r4c, half):
            # pbar[:, ct, half] (+)= pp * (1/d[c]); engines rotate so the
            # copies never gate the PE wave stream.
            dst = pbar_sb[:, ct, half * 480 : (half + 1) * 480]
            src = pp[:, 0:480]
            sc = r4c[:, ct : ct + 1]
            if h == 0:
                if ct % 2 == 0:
                    nc.scalar.activation(
                        out=dst, in_=src,
                        func=mybir.ActivationFunctionType.Copy, scale=sc,
                    )
                else:
                    nc.vector.tensor_scalar(
                        out=dst, in0=src, scalar1=sc, scalar2=None,
                        op0=mybir.AluOpType.mult,
                    )
            else:
                nc.vector.scalar_tensor_tensor(
                    out=dst, in0=src, scalar=sc, in1=dst,
                    op0=mybir.AluOpType.mult, op1=mybir.AluOpType.add,
                )

        emit_A(0)
        emit_scoresT(0)
        emit_A(1)
        # stats0's PE op (pst) goes after A1 so the PE never stalls on the
        # DVE stats chain; the chain itself overlaps A1's matmuls.
        emit_stats(0)
        emit_scoresT(1, inject=lambda jt: emit_exp(0, jt))
        emit_waves(0)
        emit_stats(1)
        for jt in range(KT):
            emit_exp(1, jt)
        emit_waves(1)
        ph2_pool.__exit__(None, None, None)

        # ---- phase 3: Z = Pbar @ Wo.T/4; y = emb_all @ Z --------------------
        p3_pool = tc.tile_pool(name="p3ps", bufs=1, space="PSUM")
        ps = p3_pool.__enter__()

        # ea.T chunks via PE transposes; first two s-tiles are hoisted ahead
        # of Z so the PE has work while the last Pv copy-outs drain.
        trts_by_st = {}

        def emit_tr(st, kt):
            kp = _kp(kt)
            at2 = trts_by_st[st]["at2"]
            ptc = ps.tile([128, 128], BF16, tag="ptc", bufs=3, name=f"ptc{st}{kt}")
            nc.tensor.transpose(
                ptc[:kp, :], at2[:, kt * 128 : kt * 128 + kp], ident[:]
            )
            trt = trp.tile([128, 128], BF16, tag="tr", name=f"trt{st}{kt}")
            if kt % 2 == 0:
                nc.vector.tensor_copy(out=trt[:kp, :], in_=ptc[:kp, :])
            else:
                nc.scalar.copy(out=trt[:kp, :], in_=ptc[:kp, :])
            trts_by_st[st]["trts"].append(trt)

        def start_tile(st):
            at2 = eap.tile([128, KV], BF16, tag="ea", name=f"at2_{st}")
            nc.sync.dma_start(
                out=at2[:], in_=ea_d.ap()[st * 128 : (st + 1) * 128, :]
            )
            trts_by_st[st] = {"at2": at2, "trts": []}

        for st in range(2):
            start_tile(st)
            for kt in range(KT):
                emit_tr(st, kt)

        # Z: wave_A halves (kt 4..7) first -- their Pbar columns are final
        # before wave_B's, so Z starts while the last copy-outs drain.
        z_sb = bigp.tile([128, KT, C], BF16, tag="big")
        for kt in [4, 5, 6, 7, 0, 1, 2, 3]:
            kp = _kp(kt)
            pz = ps.tile([128, C], F32, tag="pz", bufs=2, name=f"pz{kt}")
            for ct in range(CT):
                nc.tensor.matmul(
                    pz[:kp, :],
                    pbar_sb[:, ct, kt * 128 : kt * 128 + kp],
                    wot_sb[:, ct, :],
                    start=(ct == 0),
                    stop=(ct == CT - 1),
                )
            if kt % 2 == 0:
                nc.vector.tensor_copy(out=z_sb[:kp, kt, :], in_=pz[:kp, :])
            else:
                nc.scalar.copy(out=z_sb[:kp, kt, :], in_=pz[:kp, :])

        # y rows: transposes run two chunks ahead of the consuming matmuls so
        # the PE never waits on the PSUM->SBUF copy between them.
        for st in range(ST):
            if st >= 2:
                start_tile(st)
                emit_tr(st, 0)
                emit_tr(st, 1)
            po = ps.tile([128, C], F32, tag="po", bufs=3, name=f"po{st}")
            trts = trts_by_st[st]["trts"]
            for kt in range(KT):
                kp = _kp(kt)
                if st >= 2 and kt + 2 < KT:
                    emit_tr(st, kt + 2)
                nc.tensor.matmul(
                    po[:],
                    trts[kt][:kp, :],
                    z_sb[:kp, kt, :],
                    start=(kt == 0),
                    stop=(kt == KT - 1),
                )
            ot = outp.tile([128, C], F32, tag="out", name=f"ot{st}")
            nc.scalar.copy(out=ot[:], in_=po[:])
            nc.sync.dma_start(out=y_d.ap()[st * 128 : (st + 1) * 128, :], in_=ot[:])
        p3_pool.__exit__(None, None, None)

    nc.compile()
    return nc


_NC = None


def _get_nc():
    global _NC
    if _NC is None:
        _NC = _build_program()
    return _NC


def _in_maps(emb, emb_all, Wq, Wk, Wv, Wo):
    bfl = ml_dtypes.bfloat16
    emb = np.asarray(emb, dtype=np.float32).astype(bfl)
    emb_all = np.asarray(emb_all, dtype=np.float32).astype(bfl)
    # the 1/H head-mean factor is folded into Wo.T
    wot = (np.asarray(Wo, dtype=np.float32).T * 0.25).astype(bfl)
    Wq = np.asarray(Wq, dtype=np.float32)
    Wk = np.asarray(Wk, dtype=np.float32)
    Wv = np.asarray(Wv, dtype=np.float32)
    wv_aug = np.zeros((H, KV, KVA), dtype=np.float32)
    wv_aug[:, :, :KV] = Wv
    wv_aug[:, :, KV] = 1.0  # denominator column
    wv_aug = wv_aug.astype(bfl)
    maps = []
    for core in range(8):
        b, g = divmod(core, 2)
        h0 = 2 * g
        maps.append(
            {
                "emb": emb[b],
                "ea": emb_all[b],
                "wqt": Wq[h0 : h0 + 2].transpose(0, 2, 1).astype(bfl),
                "wkt": Wk[h0 : h0 + 2].transpose(0, 2, 1).astype(bfl),
                "wv": wv_aug[h0 : h0 + 2],
                "wot": wot,
            }
        )
    return maps


def run(emb, emb_all, Wq, Wk, Wv, Wo, trace=False):
    nc = _get_nc()
    res = run_bass_kernel_spmd(
        nc, _in_maps(emb, emb_all, Wq, Wk, Wv, Wo), list(range(8)), trace=trace
    )
    out = np.empty((B, S, C), dtype=np.float32)
    for b in range(B):
        out[b] = res.results[2 * b]["y"] + res.results[2 * b + 1]["y"]
    return out, res


def kernel(emb, emb_all, Wq, Wk, Wv, Wo):
    out, _ = run(emb, emb_all, Wq, Wk, Wv, Wo, trace=False)
    return out


# revision 20
# speedup vs baseline: 1.0411x; 1.0411x over previous
"""Trainium2 Bass kernel for nn_Attention_1013612281902.

Reference computation (per batch b, head h):
    Q = emb @ Wq[h].T            [S,C]
    K = emb_all @ Wk[h].T        [S,KV]
    V = emb_all @ Wv[h].T        [S,KV]
    scores = Q.T @ K / sqrt(KV)  [C,KV]
    normed = instance_norm(scores)       (mean/var over the whole [C,KV] plane)
    probs  = softmax(normed, axis=KV)
    context = probs @ V.T        [C,S]
    out = mean_h(context).T @ Wo.T       [S,C]

Algebraic restructuring (S=4096 >> C=512, KV=960):
    G = emb.T @ emb_all                      [C,KV]   (shared across heads)
    scores = (Wq[h] @ G @ Wk[h].T)/sqrt(KV)
    Pv[h]  = probs[h] @ Wv[h]                [C,KV]
    out    = emb_all @ (mean_h Pv[h]).T @ (Wo.T/4)
This avoids materializing Q/K/V entirely and cuts FLOPs ~4x.

All matmul operands are bfloat16 (fp32 PSUM): halves HBM traffic, enables
fast-weight-load so LDWEIGHTS hides in the PE reorder window, and runs PE
transposes at 1 cycle/row. Plane stats are computed from f32 PSUM scores;
the tiny [128,16] cross-partition stats matmul stays float32r. The Pv
matmuls put probs chunks stationary so Pv lands directly as [c,kv]
(= Pbar layout for the output projection -- no Pbar transposes), and Wv
carries an appended ones-column so each softmax denominator d[c] falls
out of the same accumulation for free. Emission order hides each head's
serial stats/exp chain under the other head's matmuls. End-to-end rel
err ~5e-3 vs the 2e-2 budget.

Sharding: 8 cores = (4 batches) x (2 head-pairs). Core 2b+g computes the
partial output for batch b over heads {2g, 2g+1}; the host adds the two
partials per batch (the head-mean and output projection are linear).
"""

import sys

if "/opt/trn_rl_repo" not in sys.path:
    sys.path.insert(0, "/opt/trn_rl_repo")

from contextlib import ExitStack

import numpy as np
import ml_dtypes

import concourse.bacc as bacc
import concourse.mybir as mybir
import concourse.tile as tile
from concourse.bass_utils import run_bass_kernel_spmd
from concourse.masks import make_identity
from concourse.tile_rust import add_dep_helper

B, S, C, KV, H = 4, 4096, 512, 960, 4
KVA = 968               # Wv free width: col 960 = ones (denominator), 961+ pad
EPS = 1e-5
F32 = mybir.dt.float32
F32R = mybir.dt.float32r
BF16 = mybir.dt.bfloat16

ST = S // 128           # 32 s-tiles
CT = C // 128           # 4 c-tiles
KT = (KV + 127) // 128  # 8 k-tiles (last one has 64 partitions)


def _kp(t):
    return min(128, KV - t * 128)


def _build_program():
    nc = bacc.Bacc("TRN2", target_bir_lowering=False, debug=False, num_devices=8)

    emb_d = nc.dram_tensor("emb", [S, C], BF16, kind="ExternalInput")
    ea_d = nc.dram_tensor("ea", [S, KV], BF16, kind="ExternalInput")
    wqt_d = nc.dram_tensor("wqt", [2, C, C], BF16, kind="ExternalInput")
    wkt_d = nc.dram_tensor("wkt", [2, KV, KV], BF16, kind="ExternalInput")
    wv_d = nc.dram_tensor("wv", [2, KV, KVA], BF16, kind="ExternalInput")
    wot_d = nc.dram_tensor("wot", [C, C], BF16, kind="ExternalInput")
    y_d = nc.dram_tensor("y", [S, C], F32, kind="ExternalOutput")

    with tile.TileContext(nc) as tc, ExitStack() as ectx:
        ec = ectx.enter_context
        const = ec(tc.tile_pool(name="const", bufs=1))
        gp = ec(tc.tile_pool(name="gp", bufs=1))
        # bufs=2 so BOTH heads' weights stream in as soon as the Sync queue
        # reaches them; with bufs=1 head-1's loads stall until head-0's
        # weights are consumed (~150us in), starving S1/W1.
        wqp = ec(tc.tile_pool(name="wqp", bufs=2))
        wkp = ec(tc.tile_pool(name="wkp", bufs=2))
        wvp = ec(tc.tile_pool(name="wvp", bufs=2))
        wop = ec(tc.tile_pool(name="wop", bufs=1))
        embp = ec(tc.tile_pool(name="embp", bufs=10))
        eap = ec(tc.tile_pool(name="eap", bufs=8))
        bigp = ec(tc.tile_pool(name="bigp", bufs=1))   # a_sb0/a_sb1/z_sb in turn
        scp = ec(tc.tile_pool(name="scp", bufs=2))     # scoresT -> probsT per head
        pbp = ec(tc.tile_pool(name="pbp", bufs=1))     # Pbar accumulator
        trp = ec(tc.tile_pool(name="trp", bufs=8))
        outp = ec(tc.tile_pool(name="outp", bufs=4))
        stp = ec(tc.tile_pool(name="stp", bufs=4))     # small stats tiles

        ident = const.tile([128, 128], BF16)
        make_identity(nc, ident[:])
        # f32r stats operand: the [128,16] cross-partition stats matmul needs
        # full fp32 precision (bf16 sums would feed var with ~0.4% error
        # straight into the softmax argument).
        onesf = const.tile([128, 128], F32)
        nc.vector.memset(onesf[:], 1.0)
        ones_r = const.tile([128, 128], F32R)
        nc.vector.tensor_copy(out=ones_r[:], in_=onesf[:])
        # scores are left unscaled (instance-norm is scale-invariant), so the
        # reference's eps applies to var/KV: use KV*eps against unscaled var.
        eps_t = const.tile([128, 1], F32)
        nc.vector.memset(eps_t[:], EPS * KV)
        # ACT-table prewarm scratch (Sqrt/Exp table loads are ~1.3us; a dummy
        # op issued early moves the load off the critical chain).
        warm = const.tile([128, 1], F32)
        nc.vector.memset(warm[:], 1.0)

        def prewarm(func, nm):
            wsink = stp.tile([128, 1], F32, tag="wsink", name=nm)
            nc.scalar.activation(out=wsink[:], in_=warm[:], func=func)

        # ---- phase 1: G = emb.T @ emb_all  [C, KV] --------------------------
        g_sb = gp.tile([128, CT, KV], BF16)
        gps_pool = tc.tile_pool(name="gps", bufs=8, space="PSUM")
        ps = gps_pool.__enter__()
        g_ps = [ps.tile([128, 480], F32, tag="ps", name=f"g_ps{i}") for i in range(8)]
        for st in range(ST):
            et = embp.tile([128, C], BF16, tag="emb", name=f"et{st}")
            at = eap.tile([128, KV], BF16, tag="ea", name=f"at{st}")
            rs = slice(st * 128, (st + 1) * 128)
            if st == 0:
                # split the first tile's loads so the first matmul (which
                # needs only et[:,0:128] + at[:,0:480]) starts ASAP.
                nc.sync.dma_start(out=at[:, 0:480], in_=ea_d.ap()[rs, 0:480])
                nc.sync.dma_start(out=et[:, 0:128], in_=emb_d.ap()[rs, 0:128])
                nc.sync.dma_start(out=et[:, 128:C], in_=emb_d.ap()[rs, 128:C])
                nc.sync.dma_start(out=at[:, 480:KV], in_=ea_d.ap()[rs, 480:KV])
            else:
                nc.sync.dma_start(out=et[:], in_=emb_d.ap()[rs, :])
                nc.sync.dma_start(out=at[:], in_=ea_d.ap()[rs, :])
            for ct in range(CT):
                for kc in range(2):
                    nc.tensor.matmul(
                        g_ps[ct * 2 + kc][:],
                        et[:, ct * 128 : (ct + 1) * 128],
                        at[:, kc * 480 : (kc + 1) * 480],
                        start=(st == 0),
                        stop=(st == ST - 1),
                    )
        for ct in range(CT):
            for kc in range(2):
                # Alternate ACT/DVE so the copy-out tail after the last G
                # matmul drains in half the time.
                dst = g_sb[:, ct, kc * 480 : (kc + 1) * 480]
                if (ct * 2 + kc) % 2 == 0:
                    nc.vector.tensor_copy(out=dst, in_=g_ps[ct * 2 + kc][:])
                else:
                    nc.scalar.copy(out=dst, in_=g_ps[ct * 2 + kc][:])
        gps_pool.__exit__(None, None, None)

        # ---- weights (host provides pre-transposed Wq.T / Wk.T / Wo.T/4) ----
        # Issued after the G-phase streams so the emb/emb_all DMAs (which
        # gate the first matmuls) get the HBM bandwidth first; within the
        # weights, in consumption order (wqt0 gates phase 2a).
        wqt_sb = []
        wkt_sb = []
        wv_sb = []
        for h in range(2):
            wq_t = wqp.tile([128, CT, C], BF16, tag="wq", name=f"wq{h}")
            nc.sync.dma_start(
                out=wq_t[:],
                in_=wqt_d.ap()[h].rearrange("(t p) d -> p t d", p=128),
            )
            wqt_sb.append(wq_t)
            # 960 rows = 7x128 + 64: two DMAs per tensor (fewer dma_starts --
            # each costs ~700ns of serial Sync-engine issue time).
            wk_t = wkp.tile([128, KT, KV], BF16, tag="wk", name=f"wk{h}")
            wv_t = wvp.tile([128, KT, KVA], BF16, tag="wv", name=f"wv{h}")
            nc.sync.dma_start(
                out=wk_t[:, 0:7, :],
                in_=wkt_d.ap()[h, 0:896, :].rearrange("(t p) d -> p t d", p=128),
            )
            nc.sync.dma_start(
                out=wk_t[:64, 7, :], in_=wkt_d.ap()[h, 896:KV, :]
            )
            nc.sync.dma_start(
                out=wv_t[:, 0:7, :],
                in_=wv_d.ap()[h, 0:896, :].rearrange("(t p) d -> p t d", p=128),
            )
            nc.sync.dma_start(
                out=wv_t[:64, 7, :], in_=wv_d.ap()[h, 896:KV, :]
            )
            wkt_sb.append(wk_t)
            wv_sb.append(wv_t)
        wot_sb = wop.tile([128, CT, C], BF16)
        nc.sync.dma_start(
            out=wot_sb[:], in_=wot_d.ap().rearrange("(t p) d -> p t d", p=128)
        )

        # ---- phase 2: per-head scores -> instancenorm -> softmax -> Pv ------
        # Emission order: A0 S0 stats0 A1 S1(+exp0 injected) waves0
        # stats1+exps1 waves1. Each head's serial stats/exp chain runs on
        # DVE/ACT under the other head's (or its own waves') PE matmuls, so
        # the PE stream A0 S0 A1 S1 W0 W1 never waits on it. One PSUM pool,
        # 8 banks: psa(2, A groups + wave_B ct0/1) + pw(4, scoresT groups +
        # wave_A + wave_B ct2/3) + one(2, stats).
        pbar_sb = pbp.tile([128, CT, KV], BF16)
        ph2_pool = tc.tile_pool(name="ph2ps", bufs=1, space="PSUM")
        ps = ph2_pool.__enter__()
        hs = [{}, {}]

        def emit_A(h):
            d = hs[h]
            d["a_sb"] = a_sb = bigp.tile(
                [128, KT, C], BF16, tag="big", name=f"a_sb{h}"
            )
            for kt in range(KT):
                kp = _kp(kt)
                pa = ps.tile([128, C], F32, tag="psa", bufs=2, name=f"pa{h}{kt}")
                for ct in range(CT):
                    nc.tensor.matmul(
                        pa[:kp, :],
                        g_sb[:, ct, kt * 128 : kt * 128 + kp],
                        wqt_sb[h][:, ct, :],
                        start=(ct == 0),
                        stop=(ct == CT - 1),
                    )
                nc.vector.tensor_copy(out=a_sb[:kp, kt, :], in_=pa[:kp, :])

        def emit_scoresT(h, inject=None):
            # scoresT[j, d] = sum_k WkT[k,j] A.T[k,d]; the reference's
            # 1/sqrt(KV) scale cancels through instance-norm (eps adjusted).
            # Per-jt stats partials (row-sum on DVE, square-sum via an
            # in-place DVE multiply-reduce -- no ACT Square table) run right
            # behind each group; `inject` emits the other head's exp ops
            # into the ACT stream so they hide under this head's matmuls.
            d = hs[h]
            a_sb = d["a_sb"]
            d["sc_sb"] = sc_sb = scp.tile(
                [128, KT, C], BF16, tag="sc", name=f"sc_sb{h}"
            )
            d["p_sb"] = p_sb = stp.tile([128, 16], F32, tag="p16", name=f"p_sb{h}")
            nc.vector.memset(p_sb[:], 0.0)
            prev_stop = None
            for jt in range(KT):
                jp = _kp(jt)
                pss = ps.tile([128, C], F32, tag="pw", bufs=4, name=f"pss{h}{jt}")
                for kt in range(KT):
                    kp = _kp(kt)
                    mm = nc.tensor.matmul(
                        pss[:jp, :],
                        wkt_sb[h][:kp, kt, jt * 128 : jt * 128 + jp],
                        a_sb[:kp, kt, :],
                        start=(kt == 0),
                        stop=(kt == KT - 1),
                    )
                    # Keep the PE stream jt-group-major: otherwise the
                    # scheduler interleaves the groups and every stop lands
                    # at the tail, stalling the stats.
                    if kt == 0 and prev_stop is not None:
                        add_dep_helper(
                            mm.ins, prev_stop.ins, sync=False, reason="jt order"
                        )
                    if kt == KT - 1:
                        prev_stop = mm
                # copy-out fused with the row-sum on ACT (Copy needs no table,
                # so the other head's exps interleave freely); square-sum as
                # pss * sc_sb (f32 PSUM x its bf16 copy) on the DVE.
                nc.scalar.activation(
                    out=sc_sb[:jp, jt, :],
                    in_=pss[:jp, :],
                    func=mybir.ActivationFunctionType.Copy,
                    accum_out=p_sb[:jp, jt : jt + 1],
                )
                sq_sink = stp.tile(
                    [128, C], BF16, tag="sqsink", name=f"sqs{h}{jt}"
                )
                nc.vector.tensor_mul(
                    out=sq_sink[:jp, :],
                    in0=pss[:jp, :],
                    in1=sc_sb[:jp, jt, :],
                )
                nc.vector.reduce_sum(
                    out=p_sb[:jp, 8 + jt : 9 + jt],
                    in_=sq_sink[:jp, :],
                    axis=mybir.AxisListType.X,
                )
                if inject is not None:
                    inject(jt)

        def emit_stats(h):
            # cross-partition reduce + broadcast of the plane stats, ending
            # in rstd / -mean*rstd for the fused exp. Runs under the next
            # phase's matmuls; prewarm(Exp) drags the table load off-chain.
            d = hs[h]
            p_sb = d["p_sb"]
            p_r = stp.tile([128, 16], F32R, tag="p16r", name=f"p_r{h}")
            nc.vector.tensor_copy(out=p_r[:], in_=p_sb[:])
            pst = ps.tile([128, 16], F32, tag="one", bufs=2, name=f"pst{h}")
            nc.tensor.matmul(pst[:], ones_r[:], p_r[:], start=True, stop=True)
            n_inv = 1.0 / float(C * KV)
            sq2 = stp.tile([128, 2], F32, tag="sq2", name=f"sq2{h}")
            nc.vector.reduce_sum(
                out=sq2[:],
                in_=pst[:].rearrange("p (a b) -> p a b", a=2),
                axis=mybir.AxisListType.X,
            )
            mean_neg = stp.tile([128, 1], F32, tag="mean", name=f"mean{h}")
            nc.vector.tensor_scalar(
                out=mean_neg[:], in0=sq2[:, 0:1], scalar1=-n_inv, scalar2=None,
                op0=mybir.AluOpType.mult,
            )
            em2 = stp.tile([128, 1], F32, tag="em2", name=f"em2{h}")
            nc.vector.tensor_scalar(
                out=em2[:], in0=sq2[:, 1:2], scalar1=n_inv, scalar2=None,
                op0=mybir.AluOpType.mult,
            )
            m2 = stp.tile([128, 1], F32, tag="m2", name=f"m2{h}")
            nc.vector.tensor_mul(out=m2[:], in0=mean_neg[:], in1=mean_neg[:])
            var_t = stp.tile([128, 1], F32, tag="var", name=f"var{h}")
            nc.vector.tensor_sub(out=var_t[:], in0=em2[:], in1=m2[:])
            std_t = stp.tile([128, 1], F32, tag="std", name=f"std{h}")
            nc.scalar.activation(
                out=std_t[:],
                in_=var_t[:],
                func=mybir.ActivationFunctionType.Sqrt,
                bias=eps_t[:],
            )
            prewarm(mybir.ActivationFunctionType.Exp, f"wex{h}")
            rstd_t = stp.tile([128, 1], F32, tag="rstd", name=f"rstd{h}")
            nc.vector.reciprocal(out=rstd_t[:], in_=std_t[:])
            negmr = stp.tile([128, 1], F32, tag="negmr", name=f"negmr{h}")
            nc.vector.tensor_mul(out=negmr[:], in0=mean_neg[:], in1=rstd_t[:])
            d["rstd"] = rstd_t
            d["negmr"] = negmr

        def emit_exp(h, jt):
            d = hs[h]
            jp = _kp(jt)
            nc.scalar.activation(
                out=d["sc_sb"][:jp, jt, :],
                in_=d["sc_sb"][:jp, jt, :],
                func=mybir.ActivationFunctionType.Exp,
                bias=d["negmr"][:jp],
                scale=d["rstd"][:jp],
            )

        def emit_waves(h):
            # Pv with probs chunks stationary: pp[ct] = sum_jt
            # probsT[jt,ct-chunk].T @ Wv-rows[jt, slice]  ->  Pv[c, kv].
            # wave_A covers kv 480:960 plus the ones column, so pp_A[:,480]
            # is the softmax denominator d[c]; its reciprocal scales every
            # copy-out. ct-major groups: d[ct] is ready as soon as group ct
            # stops, so copy-outs overlap the remaining groups.
            d = hs[h]
            sc_sb = d["sc_sb"]
            r4c = stp.tile([128, 4], F32, tag="r4c", name=f"r4c{h}")
            pp_A = []
            for ct in range(CT):
                pp = ps.tile([128, 488], F32, tag="pw", bufs=4, name=f"pA{h}{ct}")
                for jt in range(KT):
                    jp = _kp(jt)
                    nc.tensor.matmul(
                        pp[:, :],
                        sc_sb[:jp, jt, ct * 128 : (ct + 1) * 128],
                        wv_sb[h][:jp, jt, 480:KVA],
                        start=(jt == 0),
                        stop=(jt == KT - 1),
                    )
                pp_A.append(pp)
                nc.vector.reciprocal(out=r4c[:, ct : ct + 1], in_=pp[:, 480:481])
                if ct >= 2:
                    pv_out(h, ct - 2, pp_A[ct - 2], r4c, 1)
            pv_out(h, 2, pp_A[2], r4c, 1)
            pv_out(h, 3, pp_A[3], r4c, 1)
            pp_B = []
            for ct in range(CT):
                tag = "psa" if ct < 2 else "pw"
                bufs = 2 if ct < 2 else 4
                pp = ps.tile([128, 480], F32, tag=tag, bufs=bufs, name=f"pB{h}{ct}")
                for jt in range(KT):
                    jp = _kp(jt)
                    nc.tensor.matmul(
                        pp[:, :],
                        sc_sb[:jp, jt, ct * 128 : (ct + 1) * 128],
                        wv_sb[h][:jp, jt, 0:480],
                        start=(jt == 0),
                        stop=(jt == KT - 1),
                    )
                pp_B.append(pp)
                if ct >= 1:
                    pv_out(h, ct - 1, pp_B[ct - 1], r4c, 0)
            pv_out(h, 3, pp_B[3], r4c, 0)

        def pv_out(h, ct, pp, r4c, half):
            # pbar[:, ct, half] (+)= pp * (1/d[c]); engines rotate so the
            # copies never gate the PE wave stream.
            # all on DVE: an ACT copy here would sit in the ACT FIFO in front
            # of the next head's stats/exp chain while waiting on this wave's
            # stop, stalling the following wave by ~5us (head-of-line block).
            dst = pbar_sb[:, ct, half * 480 : (half + 1) * 480]
            src = pp[:, 0:480]
            sc = r4c[:, ct : ct + 1]
            if h == 0:
                nc.vector.tensor_scalar(
                    out=dst, in0=src, scalar1=sc, scalar2=None,
                    op0=mybir.AluOpType.mult,
                )
            else:
                nc.vector.scalar_tensor_tensor(
                    out=dst, in0=src, scalar=sc, in1=dst,
                    op0=mybir.AluOpType.mult, op1=mybir.AluOpType.add,
                )

        emit_A(0)
        emit_scoresT(0)
        emit_A(1)
        # stats0's PE op (pst) goes after A1 so the PE never stalls on the
        # DVE stats chain; the chain itself overlaps A1's matmuls.
        emit_stats(0)
        emit_scoresT(1, inject=lambda jt: emit_exp(0, jt))
        emit_waves(0)
        emit_stats(1)
        for jt in range(KT):
            emit_exp(1, jt)
        emit_waves(1)
        ph2_pool.__exit__(None, None, None)

        # ---- phase 3: Z = Pbar @ Wo.T/4; y = emb_all @ Z --------------------
        p3_pool = tc.tile_pool(name="p3ps", bufs=1, space="PSUM")
        ps = p3_pool.__enter__()

        # ea.T chunks via PE transposes, batched four-to-a-PSUM-tile so one
        # DVE copy moves four chunks; transposes run two s-tiles ahead of
        # their consuming matmuls (and the first two tiles ahead of Z, so the
        # PE has work while the last Pv copy-outs drain).
        trts_by_st = {}

        def start_tile(st):
            at2 = eap.tile([128, KV], BF16, tag="ea", name=f"at2_{st}")
            nc.sync.dma_start(
                out=at2[:], in_=ea_d.ap()[st * 128 : (st + 1) * 128, :]
            )
            trts_by_st[st] = {"at2": at2, "halves": []}

        def emit_tr_half(st, half):
            # one PSUM tile per chunk: a matmul/transpose output must start
            # at a PSUM bank base (sub-bank offsets hard-fault the device).
            at2 = trts_by_st[st]["at2"]
            trt = trp.tile([128, 512], BF16, tag="tr", name=f"trt{st}{half}")
            for k in range(4):
                kt = half * 4 + k
                kp = _kp(kt)
                ptc = ps.tile(
                    [128, 128], BF16, tag="ptc", bufs=4, name=f"ptc{st}{kt}"
                )
                nc.tensor.transpose(
                    ptc[:kp, :], at2[:, kt * 128 : kt * 128 + kp], ident[:]
                )
                nc.vector.tensor_copy(
                    out=trt[:kp, k * 128 : k * 128 + 128], in_=ptc[:kp, :]
                )
            trts_by_st[st]["halves"].append(trt)

        for st in range(4):
            start_tile(st)
        for st in range(2):
            emit_tr_half(st, 0)
            emit_tr_half(st, 1)

        # Z: wave_A halves (kt 4..7) first -- their Pbar columns are final
        # before wave_B's, so Z starts while the last copy-outs drain.
        z_sb = bigp.tile([128, KT, C], BF16, tag="big")
        for kt in [4, 5, 6, 7, 0, 1, 2, 3]:
            kp = _kp(kt)
            pz = ps.tile([128, C], F32, tag="pz", bufs=2, name=f"pz{kt}")
            for ct in range(CT):
                nc.tensor.matmul(
                    pz[:kp, :],
                    pbar_sb[:, ct, kt * 128 : kt * 128 + kp],
                    wot_sb[:, ct, :],
                    start=(ct == 0),
                    stop=(ct == CT - 1),
                )
            if kt % 2 == 0:
                nc.vector.tensor_copy(out=z_sb[:kp, kt, :], in_=pz[:kp, :])
            else:
                nc.scalar.copy(out=z_sb[:kp, kt, :], in_=pz[:kp, :])

        # y rows: each iteration runs this tile's matmuls while emitting the
        # transposes (and input DMA) for tile st+2 between them.
        for st in range(ST):
            if st + 4 < ST:
                start_tile(st + 4)
            po = ps.tile([128, C], F32, tag="po", bufs=2, name=f"po{st}")
            halves = trts_by_st[st]["halves"]
            for kt in range(KT):
                kp = _kp(kt)
                if st + 2 < ST and kt == 1:
                    emit_tr_half(st + 2, 0)
                if st + 2 < ST and kt == 5:
                    emit_tr_half(st + 2, 1)
                nc.tensor.matmul(
                    po[:],
                    halves[kt // 4][:kp, (kt % 4) * 128 : (kt % 4) * 128 + 128],
                    z_sb[:kp, kt, :],
                    start=(kt == 0),
                    stop=(kt == KT - 1),
                )
            ot = outp.tile([128, C], F32, tag="out", name=f"ot{st}")
            nc.scalar.copy(out=ot[:], in_=po[:])
            nc.sync.dma_start(out=y_d.ap()[st * 128 : (st + 1) * 128, :], in_=ot[:])
        p3_pool.__exit__(None, None, None)

    nc.compile()
    return nc


_NC = None


def _get_nc():
    global _NC
    if _NC is None:
        _NC = _build_program()
    return _NC


def _in_maps(emb, emb_all, Wq, Wk, Wv, Wo):
    bfl = ml_dtypes.bfloat16
    emb = np.asarray(emb, dtype=np.float32).astype(bfl)
    emb_all = np.asarray(emb_all, dtype=np.float32).astype(bfl)
    # the 1/H head-mean factor is folded into Wo.T
    wot = (np.asarray(Wo, dtype=np.float32).T * 0.25).astype(bfl)
    Wq = np.asarray(Wq, dtype=np.float32)
    Wk = np.asarray(Wk, dtype=np.float32)
    Wv = np.asarray(Wv, dtype=np.float32)
    wv_aug = np.zeros((H, KV, KVA), dtype=np.float32)
    wv_aug[:, :, :KV] = Wv
    wv_aug[:, :, KV] = 1.0  # denominator column
    wv_aug = wv_aug.astype(bfl)
    maps = []
    for core in range(8):
        b, g = divmod(core, 2)
        h0 = 2 * g
        maps.append(
            {
                "emb": emb[b],
                "ea": emb_all[b],
                "wqt": Wq[h0 : h0 + 2].transpose(0, 2, 1).astype(bfl),
                "wkt": Wk[h0 : h0 + 2].transpose(0, 2, 1).astype(bfl),
                "wv": wv_aug[h0 : h0 + 2],
                "wot": wot,
            }
        )
    return maps


def run(emb, emb_all, Wq, Wk, Wv, Wo, trace=False):
    nc = _get_nc()
    res = run_bass_kernel_spmd(
        nc, _in_maps(emb, emb_all, Wq, Wk, Wv, Wo), list(range(8)), trace=trace
    )
    out = np.empty((B, S, C), dtype=np.float32)
    for b in range(B):
        out[b] = res.results[2 * b]["y"] + res.results[2 * b + 1]["y"]
    return out, res


def kernel(emb, emb_all, Wq, Wk, Wv, Wo):
    out, _ = run(emb, emb_all, Wq, Wk, Wv, Wo, trace=False)
    return out


# revision 22
# speedup vs baseline: 1.2798x; 1.2293x over previous
"""Trainium2 Bass kernel for nn_Attention_1013612281902.

Reference computation (per batch b, head h):
    Q = emb @ Wq[h].T            [S,C]
    K = emb_all @ Wk[h].T        [S,KV]
    V = emb_all @ Wv[h].T        [S,KV]
    scores = Q.T @ K / sqrt(KV)  [C,KV]
    normed = instance_norm(scores)       (mean/var over the whole [C,KV] plane)
    probs  = softmax(normed, axis=KV)
    context = probs @ V.T        [C,S]
    out = mean_h(context).T @ Wo.T       [S,C]

Algebraic restructuring (S=4096 >> C=512, KV=960):
    G = emb.T @ emb_all                      [C,KV]   (shared across heads)
    scores = (Wq[h] @ G @ Wk[h].T)/sqrt(KV)
    Pv[h]  = probs[h] @ Wv[h]                [C,KV]
    out    = emb_all @ (mean_h Pv[h]).T @ (Wo.T/4)
This avoids materializing Q/K/V entirely and cuts FLOPs ~4x.

All matmul operands are bfloat16 (fp32 PSUM): halves HBM traffic, enables
fast-weight-load so LDWEIGHTS hides in the PE reorder window, and runs PE
transposes at 1 cycle/row. Plane stats are computed from f32 PSUM scores;
the tiny [128,16] cross-partition stats matmul stays float32r. The Pv
matmuls put probs chunks stationary so Pv lands directly as [c,kv]
(= Pbar layout for the output projection -- no Pbar transposes), and Wv
carries an appended ones-column so each softmax denominator d[c] falls
out of the same accumulation for free. Emission order hides each head's
serial stats/exp chain under the other head's matmuls. End-to-end rel
err ~5e-3 vs the 2e-2 budget.

Sharding: 8 cores = (4 batches) x (2 head-pairs). Core 2b+g computes the
partial output for batch b over heads {2g, 2g+1}; the host adds the two
partials per batch (the head-mean and output projection are linear).
"""

import sys

if "/opt/trn_rl_repo" not in sys.path:
    sys.path.insert(0, "/opt/trn_rl_repo")

from contextlib import ExitStack

import numpy as np
import ml_dtypes

import concourse.bacc as bacc
import concourse.mybir as mybir
import concourse.tile as tile
from concourse.bass_utils import run_bass_kernel_spmd
from concourse.masks import make_identity
from concourse.tile_rust import add_dep_helper

B, S, C, KV, H = 4, 4096, 512, 960, 4
KVA = 968               # Wv free width: col 960 = ones (denominator), 961+ pad
EPS = 1e-5
F32 = mybir.dt.float32
F32R = mybir.dt.float32r
BF16 = mybir.dt.bfloat16

ST = S // 128           # 32 s-tiles
CT = C // 128           # 4 c-tiles
KT = (KV + 127) // 128  # 8 k-tiles (last one has 64 partitions)


def _kp(t):
    return min(128, KV - t * 128)


def _build_program():
    nc = bacc.Bacc("TRN2", target_bir_lowering=False, debug=False, num_devices=8)

    emb_d = nc.dram_tensor("emb", [S, C], BF16, kind="ExternalInput")
    ea_d = nc.dram_tensor("ea", [S, KV], BF16, kind="ExternalInput")
    wqt_d = nc.dram_tensor("wqt", [2, C, C], BF16, kind="ExternalInput")
    wkt_d = nc.dram_tensor("wkt", [2, KV, KV], BF16, kind="ExternalInput")
    wv_d = nc.dram_tensor("wv", [2, KV, KVA], BF16, kind="ExternalInput")
    wot_d = nc.dram_tensor("wot", [C, C], BF16, kind="ExternalInput")
    y_d = nc.dram_tensor("y", [S, C], F32, kind="ExternalOutput")

    with tile.TileContext(nc) as tc, ExitStack() as ectx:
        ec = ectx.enter_context
        const = ec(tc.tile_pool(name="const", bufs=1))
        gp = ec(tc.tile_pool(name="gp", bufs=1))
        # bufs=2 so BOTH heads' weights stream in as soon as the Sync queue
        # reaches them; with bufs=1 head-1's loads stall until head-0's
        # weights are consumed (~150us in), starving S1/W1.
        wqp = ec(tc.tile_pool(name="wqp", bufs=2))
        wkp = ec(tc.tile_pool(name="wkp", bufs=2))
        wvp = ec(tc.tile_pool(name="wvp", bufs=2))
        wop = ec(tc.tile_pool(name="wop", bufs=1))
        embp = ec(tc.tile_pool(name="embp", bufs=10))
        eap = ec(tc.tile_pool(name="eap", bufs=8))
        bigp = ec(tc.tile_pool(name="bigp", bufs=1))   # a_sb0/a_sb1/z_sb in turn
        scp = ec(tc.tile_pool(name="scp", bufs=2))     # scoresT -> probsT per head
        pbp = ec(tc.tile_pool(name="pbp", bufs=1))     # Pbar accumulator
        trp = ec(tc.tile_pool(name="trp", bufs=8))
        outp = ec(tc.tile_pool(name="outp", bufs=4))
        stp = ec(tc.tile_pool(name="stp", bufs=4))     # small stats tiles

        ident = const.tile([128, 128], BF16)
        make_identity(nc, ident[:])
        # f32r stats operand: the [128,16] cross-partition stats matmul needs
        # full fp32 precision (bf16 sums would feed var with ~0.4% error
        # straight into the softmax argument).
        onesf = const.tile([128, 128], F32)
        nc.vector.memset(onesf[:], 1.0)
        ones_r = const.tile([128, 128], F32R)
        nc.vector.tensor_copy(out=ones_r[:], in_=onesf[:])
        # scores are left unscaled (instance-norm is scale-invariant), so the
        # reference's eps applies to var/KV: use KV*eps against unscaled var.
        eps_t = const.tile([128, 1], F32)
        nc.vector.memset(eps_t[:], EPS * KV)
        # ACT-table prewarm scratch (Sqrt/Exp table loads are ~1.3us; a dummy
        # op issued early moves the load off the critical chain).
        warm = const.tile([128, 1], F32)
        nc.vector.memset(warm[:], 1.0)

        def prewarm(func, nm):
            wsink = stp.tile([128, 1], F32, tag="wsink", name=nm)
            nc.scalar.activation(out=wsink[:], in_=warm[:], func=func)

        # ---- phase 1: G = emb.T @ emb_all  [C, KV] --------------------------
        g_sb = gp.tile([128, CT, KV], BF16)
        gps_pool = tc.tile_pool(name="gps", bufs=8, space="PSUM")
        ps = gps_pool.__enter__()
        g_ps = [ps.tile([128, 480], F32, tag="ps", name=f"g_ps{i}") for i in range(8)]
        for st in range(ST):
            et = embp.tile([128, C], BF16, tag="emb", name=f"et{st}")
            at = eap.tile([128, KV], BF16, tag="ea", name=f"at{st}")
            rs = slice(st * 128, (st + 1) * 128)
            if st == 0:
                # split the first tile's loads so the first matmul (which
                # needs only et[:,0:128] + at[:,0:480]) starts ASAP.
                nc.sync.dma_start(out=at[:, 0:480], in_=ea_d.ap()[rs, 0:480])
                nc.sync.dma_start(out=et[:, 0:128], in_=emb_d.ap()[rs, 0:128])
                nc.sync.dma_start(out=et[:, 128:C], in_=emb_d.ap()[rs, 128:C])
                nc.sync.dma_start(out=at[:, 480:KV], in_=ea_d.ap()[rs, 480:KV])
            else:
                nc.sync.dma_start(out=et[:], in_=emb_d.ap()[rs, :])
                nc.sync.dma_start(out=at[:], in_=ea_d.ap()[rs, :])
            for ct in range(CT):
                for kc in range(2):
                    nc.tensor.matmul(
                        g_ps[ct * 2 + kc][:],
                        et[:, ct * 128 : (ct + 1) * 128],
                        at[:, kc * 480 : (kc + 1) * 480],
                        start=(st == 0),
                        stop=(st == ST - 1),
                    )
        for ct in range(CT):
            for kc in range(2):
                # Alternate ACT/DVE so the copy-out tail after the last G
                # matmul drains in half the time.
                dst = g_sb[:, ct, kc * 480 : (kc + 1) * 480]
                if (ct * 2 + kc) % 2 == 0:
                    nc.vector.tensor_copy(out=dst, in_=g_ps[ct * 2 + kc][:])
                else:
                    nc.scalar.copy(out=dst, in_=g_ps[ct * 2 + kc][:])
        gps_pool.__exit__(None, None, None)

        # ---- weights (host provides pre-transposed Wq.T / Wk.T / Wo.T/4) ----
        # Issued after the G-phase streams so the emb/emb_all DMAs (which
        # gate the first matmuls) get the HBM bandwidth first; within the
        # weights, in consumption order (wqt0 gates phase 2a).
        wqt_sb = []
        wkt_sb = []
        wv_sb = []
        for h in range(2):
            wq_t = wqp.tile([128, CT, C], BF16, tag="wq", name=f"wq{h}")
            nc.sync.dma_start(
                out=wq_t[:],
                in_=wqt_d.ap()[h].rearrange("(t p) d -> p t d", p=128),
            )
            wqt_sb.append(wq_t)
            # 960 rows = 7x128 + 64: two DMAs per tensor (fewer dma_starts --
            # each costs ~700ns of serial Sync-engine issue time).
            wk_t = wkp.tile([128, KT, KV], BF16, tag="wk", name=f"wk{h}")
            wv_t = wvp.tile([128, KT, KVA], BF16, tag="wv", name=f"wv{h}")
            nc.sync.dma_start(
                out=wk_t[:, 0:7, :],
                in_=wkt_d.ap()[h, 0:896, :].rearrange("(t p) d -> p t d", p=128),
            )
            nc.sync.dma_start(
                out=wk_t[:64, 7, :], in_=wkt_d.ap()[h, 896:KV, :]
            )
            nc.sync.dma_start(
                out=wv_t[:, 0:7, :],
                in_=wv_d.ap()[h, 0:896, :].rearrange("(t p) d -> p t d", p=128),
            )
            nc.sync.dma_start(
                out=wv_t[:64, 7, :], in_=wv_d.ap()[h, 896:KV, :]
            )
            wkt_sb.append(wk_t)
            wv_sb.append(wv_t)
        wot_sb = wop.tile([128, CT, C], BF16)
        nc.sync.dma_start(
            out=wot_sb[:], in_=wot_d.ap().rearrange("(t p) d -> p t d", p=128)
        )

        # ---- phase 2: per-head scores -> instancenorm -> softmax -> Pv ------
        # Emission order: A0 S0 stats0 A1 S1(+exp0 injected) waves0
        # stats1+exps1 waves1. Each head's serial stats/exp chain runs on
        # DVE/ACT under the other head's (or its own waves') PE matmuls, so
        # the PE stream A0 S0 A1 S1 W0 W1 never waits on it. One PSUM pool,
        # 8 banks: psa(2, A groups + wave_B ct0/1) + pw(4, scoresT groups +
        # wave_A + wave_B ct2/3) + one(2, stats).
        pbar_sb = pbp.tile([128, CT, KV], BF16)
        ph2_pool = tc.tile_pool(name="ph2ps", bufs=1, space="PSUM")
        ps = ph2_pool.__enter__()
        hs = [{}, {}]

        def emit_A(h):
            d = hs[h]
            d["a_sb"] = a_sb = bigp.tile(
                [128, KT, C], BF16, tag="big", name=f"a_sb{h}"
            )
            for kt in range(KT):
                kp = _kp(kt)
                pa = ps.tile([128, C], F32, tag="psa", bufs=2, name=f"pa{h}{kt}")
                for ct in range(CT):
                    nc.tensor.matmul(
                        pa[:kp, :],
                        g_sb[:, ct, kt * 128 : kt * 128 + kp],
                        wqt_sb[h][:, ct, :],
                        start=(ct == 0),
                        stop=(ct == CT - 1),
                    )
                nc.vector.tensor_copy(out=a_sb[:kp, kt, :], in_=pa[:kp, :])

        def emit_scoresT(h, inject=None):
            # scoresT[j, d] = sum_k WkT[k,j] A.T[k,d]; the reference's
            # 1/sqrt(KV) scale cancels through instance-norm (eps adjusted).
            # Per-jt stats partials (row-sum on DVE, square-sum via an
            # in-place DVE multiply-reduce -- no ACT Square table) run right
            # behind each group; `inject` emits the other head's exp ops
            # into the ACT stream so they hide under this head's matmuls.
            d = hs[h]
            a_sb = d["a_sb"]
            d["sc_sb"] = sc_sb = scp.tile(
                [128, KT, C], BF16, tag="sc", name=f"sc_sb{h}"
            )
            d["p_sb"] = p_sb = stp.tile([128, 16], F32, tag="p16", name=f"p_sb{h}")
            nc.vector.memset(p_sb[:], 0.0)
            prev_stop = None
            for jt in range(KT):
                jp = _kp(jt)
                pss = ps.tile([128, C], F32, tag="pw", bufs=4, name=f"pss{h}{jt}")
                for kt in range(KT):
                    kp = _kp(kt)
                    mm = nc.tensor.matmul(
                        pss[:jp, :],
                        wkt_sb[h][:kp, kt, jt * 128 : jt * 128 + jp],
                        a_sb[:kp, kt, :],
                        start=(kt == 0),
                        stop=(kt == KT - 1),
                    )
                    # Keep the PE stream jt-group-major: otherwise the
                    # scheduler interleaves the groups and every stop lands
                    # at the tail, stalling the stats.
                    if kt == 0 and prev_stop is not None:
                        add_dep_helper(
                            mm.ins, prev_stop.ins, sync=False, reason="jt order"
                        )
                    if kt == KT - 1:
                        prev_stop = mm
                # copy-out fused with the row-sum on ACT (Copy needs no table,
                # so the other head's exps interleave freely); square-sum as
                # pss * sc_sb (f32 PSUM x its bf16 copy) on the DVE.
                nc.scalar.activation(
                    out=sc_sb[:jp, jt, :],
                    in_=pss[:jp, :],
                    func=mybir.ActivationFunctionType.Copy,
                    accum_out=p_sb[:jp, jt : jt + 1],
                )
                sq_sink = stp.tile(
                    [128, C], BF16, tag="sqsink", name=f"sqs{h}{jt}"
                )
                nc.vector.tensor_mul(
                    out=sq_sink[:jp, :],
                    in0=pss[:jp, :],
                    in1=sc_sb[:jp, jt, :],
                )
                nc.vector.reduce_sum(
                    out=p_sb[:jp, 8 + jt : 9 + jt],
                    in_=sq_sink[:jp, :],
                    axis=mybir.AxisListType.X,
                )
                if inject is not None:
                    inject(jt)

        def emit_stats(h):
            # cross-partition reduce + broadcast of the plane stats, ending
            # in rstd / -mean*rstd for the fused exp. Runs under the next
            # phase's matmuls; prewarm(Exp) drags the table load off-chain.
            d = hs[h]
            p_sb = d["p_sb"]
            p_r = stp.tile([128, 16], F32R, tag="p16r", name=f"p_r{h}")
            nc.vector.tensor_copy(out=p_r[:], in_=p_sb[:])
            pst = ps.tile([128, 16], F32, tag="one", bufs=2, name=f"pst{h}")
            nc.tensor.matmul(pst[:], ones_r[:], p_r[:], start=True, stop=True)
            n_inv = 1.0 / float(C * KV)
            sq2 = stp.tile([128, 2], F32, tag="sq2", name=f"sq2{h}")
            nc.vector.reduce_sum(
                out=sq2[:],
                in_=pst[:].rearrange("p (a b) -> p a b", a=2),
                axis=mybir.AxisListType.X,
            )
            mean_neg = stp.tile([128, 1], F32, tag="mean", name=f"mean{h}")
            nc.vector.tensor_scalar(
                out=mean_neg[:], in0=sq2[:, 0:1], scalar1=-n_inv, scalar2=None,
                op0=mybir.AluOpType.mult,
            )
            em2 = stp.tile([128, 1], F32, tag="em2", name=f"em2{h}")
            nc.vector.tensor_scalar(
                out=em2[:], in0=sq2[:, 1:2], scalar1=n_inv, scalar2=None,
                op0=mybir.AluOpType.mult,
            )
            m2 = stp.tile([128, 1], F32, tag="m2", name=f"m2{h}")
            nc.vector.tensor_mul(out=m2[:], in0=mean_neg[:], in1=mean_neg[:])
            var_t = stp.tile([128, 1], F32, tag="var", name=f"var{h}")
            nc.vector.tensor_sub(out=var_t[:], in0=em2[:], in1=m2[:])
            std_t = stp.tile([128, 1], F32, tag="std", name=f"std{h}")
            nc.scalar.activation(
                out=std_t[:],
                in_=var_t[:],
                func=mybir.ActivationFunctionType.Sqrt,
                bias=eps_t[:],
            )
            prewarm(mybir.ActivationFunctionType.Exp, f"wex{h}")
            rstd_t = stp.tile([128, 1], F32, tag="rstd", name=f"rstd{h}")
            nc.vector.reciprocal(out=rstd_t[:], in_=std_t[:])
            negmr = stp.tile([128, 1], F32, tag="negmr", name=f"negmr{h}")
            nc.vector.tensor_mul(out=negmr[:], in0=mean_neg[:], in1=rstd_t[:])
            d["rstd"] = rstd_t
            d["negmr"] = negmr

        def emit_exp(h, jt):
            d = hs[h]
            jp = _kp(jt)
            nc.scalar.activation(
                out=d["sc_sb"][:jp, jt, :],
                in_=d["sc_sb"][:jp, jt, :],
                func=mybir.ActivationFunctionType.Exp,
                bias=d["negmr"][:jp],
                scale=d["rstd"][:jp],
            )

        def emit_waves(h):
            # Pv with probs chunks stationary: pp[ct] = sum_jt
            # probsT[jt,ct-chunk].T @ Wv-rows[jt, slice]  ->  Pv[c, kv].
            # wave_A covers kv 480:960 plus the ones column, so pp_A[:,480]
            # is the softmax denominator d[c]; its reciprocal scales every
            # copy-out. ct-major groups: d[ct] is ready as soon as group ct
            # stops, so copy-outs overlap the remaining groups.
            d = hs[h]
            sc_sb = d["sc_sb"]
            r4c = stp.tile([128, 4], F32, tag="r4c", name=f"r4c{h}")
            pp_A = []
            for ct in range(CT):
                pp = ps.tile([128, 488], F32, tag="pw", bufs=4, name=f"pA{h}{ct}")
                for jt in range(KT):
                    jp = _kp(jt)
                    nc.tensor.matmul(
                        pp[:, :],
                        sc_sb[:jp, jt, ct * 128 : (ct + 1) * 128],
                        wv_sb[h][:jp, jt, 480:KVA],
                        start=(jt == 0),
                        stop=(jt == KT - 1),
                    )
                pp_A.append(pp)
                nc.vector.reciprocal(out=r4c[:, ct : ct + 1], in_=pp[:, 480:481])
                if ct >= 2:
                    pv_out(h, ct - 2, pp_A[ct - 2], r4c, 1)
            pv_out(h, 2, pp_A[2], r4c, 1)
            pv_out(h, 3, pp_A[3], r4c, 1)
            pp_B = []
            for ct in range(CT):
                tag = "psa" if ct < 2 else "pw"
                bufs = 2 if ct < 2 else 4
                pp = ps.tile([128, 480], F32, tag=tag, bufs=bufs, name=f"pB{h}{ct}")
                for jt in range(KT):
                    jp = _kp(jt)
                    nc.tensor.matmul(
                        pp[:, :],
                        sc_sb[:jp, jt, ct * 128 : (ct + 1) * 128],
                        wv_sb[h][:jp, jt, 0:480],
                        start=(jt == 0),
                        stop=(jt == KT - 1),
                    )
                pp_B.append(pp)
                if ct >= 1:
                    pv_out(h, ct - 1, pp_B[ct - 1], r4c, 0)
            pv_out(h, 3, pp_B[3], r4c, 0)

        def pv_out(h, ct, pp, r4c, half):
            # pbar[:, ct, half] (+)= pp * (1/d[c]); engines rotate so the
            # copies never gate the PE wave stream.
            # all on DVE: an ACT copy here would sit in the ACT FIFO in front
            # of the next head's stats/exp chain while waiting on this wave's
            # stop, stalling the following wave by ~5us (head-of-line block).
            dst = pbar_sb[:, ct, half * 480 : (half + 1) * 480]
            src = pp[:, 0:480]
            sc = r4c[:, ct : ct + 1]
            if h == 0:
                nc.vector.tensor_scalar(
                    out=dst, in0=src, scalar1=sc, scalar2=None,
                    op0=mybir.AluOpType.mult,
                )
            else:
                nc.vector.scalar_tensor_tensor(
                    out=dst, in0=src, scalar=sc, in1=dst,
                    op0=mybir.AluOpType.mult, op1=mybir.AluOpType.add,
                )

        # ea.T transpose machinery (used both inside phase 2 and in phase 3).
        trts_by_st = {}

        def start_tile(st):
            at2 = eap.tile([128, KV], BF16, tag="ea", name=f"at2_{st}")
            nc.sync.dma_start(
                out=at2[:], in_=ea_d.ap()[st * 128 : (st + 1) * 128, :]
            )
            trts_by_st[st] = {"at2": at2, "halves": []}

        def emit_tr_half(st, half, pool, tag, bufs):
            # one PSUM tile per chunk: a matmul/transpose output must start
            # at a PSUM bank base (sub-bank offsets hard-fault the device).
            at2 = trts_by_st[st]["at2"]
            trt = trp.tile([128, 512], BF16, tag="tr", name=f"trt{st}{half}")
            for k in range(4):
                kt = half * 4 + k
                kp = _kp(kt)
                ptc = pool.tile(
                    [128, 128], BF16, tag=tag, bufs=bufs, name=f"ptc{st}{kt}"
                )
                nc.tensor.transpose(
                    ptc[:kp, :], at2[:, kt * 128 : kt * 128 + kp], ident[:]
                )
                nc.vector.tensor_copy(
                    out=trt[:kp, k * 128 : k * 128 + 128], in_=ptc[:kp, :]
                )
            trts_by_st[st]["halves"].append(trt)

        emit_A(0)
        emit_scoresT(0)
        emit_A(1)
        # stats0's PE op (pst) goes after A1 so the PE never stalls on the
        # DVE stats chain; the chain itself overlaps A1's matmuls.
        emit_stats(0)
        emit_scoresT(1, inject=lambda jt: emit_exp(0, jt))
        # The first two output-phase s-tiles transpose here, on the psa banks
        # (idle since A1): this PE work covers stats1's wait on the last S1
        # stats partials, so pst1 -> exps1 all complete during W0 and W1
        # starts unthrottled.
        start_tile(0)
        start_tile(1)
        for st in range(2):
            emit_tr_half(st, 0, ps, "psa", 2)
            emit_tr_half(st, 1, ps, "psa", 2)
        emit_stats(1)
        for jt in range(KT):
            emit_exp(1, jt)
        emit_waves(0)
        emit_waves(1)
        ph2_pool.__exit__(None, None, None)

        # ---- phase 3: Z = Pbar @ Wo.T/4; y = emb_all @ Z --------------------
        p3_pool = tc.tile_pool(name="p3ps", bufs=1, space="PSUM")
        ps = p3_pool.__enter__()

        start_tile(2)
        start_tile(3)

        # Z: wave_A halves (kt 4..7) first -- their Pbar columns are final
        # before wave_B's, so Z starts while the last copy-outs drain.
        z_sb = bigp.tile([128, KT, C], BF16, tag="big")
        for kt in [4, 5, 6, 7, 0, 1, 2, 3]:
            kp = _kp(kt)
            pz = ps.tile([128, C], F32, tag="pz", bufs=2, name=f"pz{kt}")
            for ct in range(CT):
                nc.tensor.matmul(
                    pz[:kp, :],
                    pbar_sb[:, ct, kt * 128 : kt * 128 + kp],
                    wot_sb[:, ct, :],
                    start=(ct == 0),
                    stop=(ct == CT - 1),
                )
            if kt % 2 == 0:
                nc.vector.tensor_copy(out=z_sb[:kp, kt, :], in_=pz[:kp, :])
            else:
                nc.scalar.copy(out=z_sb[:kp, kt, :], in_=pz[:kp, :])

        # y rows: each iteration runs this tile's matmuls while emitting the
        # transposes (and input DMA) for tile st+2 between them.
        for st in range(ST):
            if st + 4 < ST:
                start_tile(st + 4)
            po = ps.tile([128, C], F32, tag="po", bufs=2, name=f"po{st}")
            halves = trts_by_st[st]["halves"]
            for kt in range(KT):
                kp = _kp(kt)
                if st + 2 < ST and kt == 1:
                    emit_tr_half(st + 2, 0, ps, "ptc", 4)
                if st + 2 < ST and kt == 5:
                    emit_tr_half(st + 2, 1, ps, "ptc", 4)
                nc.tensor.matmul(
                    po[:],
                    halves[kt // 4][:kp, (kt % 4) * 128 : (kt % 4) * 128 + 128],
                    z_sb[:kp, kt, :],
                    start=(kt == 0),
                    stop=(kt == KT - 1),
                )
            ot = outp.tile([128, C], F32, tag="out", name=f"ot{st}")
            nc.scalar.copy(out=ot[:], in_=po[:])
            nc.sync.dma_start(out=y_d.ap()[st * 128 : (st + 1) * 128, :], in_=ot[:])
        p3_pool.__exit__(None, None, None)

    nc.compile()
    return nc


_NC = None


def _get_nc():
    global _NC
    if _NC is None:
        _NC = _build_program()
    return _NC


def _in_maps(emb, emb_all, Wq, Wk, Wv, Wo):
    bfl = ml_dtypes.bfloat16
    emb = np.asarray(emb, dtype=np.float32).astype(bfl)
    emb_all = np.asarray(emb_all, dtype=np.float32).astype(bfl)
    # the 1/H head-mean factor is folded into Wo.T
    wot = (np.asarray(Wo, dtype=np.float32).T * 0.25).astype(bfl)
    Wq = np.asarray(Wq, dtype=np.float32)
    Wk = np.asarray(Wk, dtype=np.float32)
    Wv = np.asarray(Wv, dtype=np.float32)
    wv_aug = np.zeros((H, KV, KVA), dtype=np.float32)
    wv_aug[:, :, :KV] = Wv
    wv_aug[:, :, KV] = 1.0  # denominator column
    wv_aug = wv_aug.astype(bfl)
    maps = []
    for core in range(8):
        b, g = divmod(core, 2)
        h0 = 2 * g
        maps.append(
            {
                "emb": emb[b],
                "ea": emb_all[b],
                "wqt": Wq[h0 : h0 + 2].transpose(0, 2, 1).astype(bfl),
                "wkt": Wk[h0 : h0 + 2].transpose(0, 2, 1).astype(bfl),
                "wv": wv_aug[h0 : h0 + 2],
                "wot": wot,
            }
        )
    return maps


def run(emb, emb_all, Wq, Wk, Wv, Wo, trace=False):
    nc = _get_nc()
    res = run_bass_kernel_spmd(
        nc, _in_maps(emb, emb_all, Wq, Wk, Wv, Wo), list(range(8)), trace=trace
    )
    out = np.empty((B, S, C), dtype=np.float32)
    for b in range(B):
        out[b] = res.results[2 * b]["y"] + res.results[2 * b + 1]["y"]
    return out, res


def kernel(emb, emb_all, Wq, Wk, Wv, Wo):
    out, _ = run(emb, emb_all, Wq, Wk, Wv, Wo, trace=False)
    return out
